# revision 2
# baseline (speedup 1.0000x reference)
"""Trainium2 Bass kernel for nn_CBlock3D: Conv3d(16->32, k=3, SAME) + BatchNorm3d
(training-mode batch stats) + softplus, on x[4,16,16,64,64] f32.

Strategy (8 NeuronCores, SPMD):
  - Shard (batch n, depth-half dh): 8 shards of [16, 8, 64, 64] output depth-slabs.
    Host pre-pads each shard's input to [16, 10, 66, 66] (zero SAME-padding in
    h/w, depth halo from neighbors) and pre-builds a kw-triplicated buffer
    xb[48, 43562] fp16 where row (kw*16+ci) is the flattened padded volume of
    channel ci shifted by kw. A single matmul AP can then read all (kw, ci)
    contraction rows with one uniform offset.
  - Conv = 9 accumulating matmuls per output tile (one per (kd, kh) tap),
    contraction K=48 = 3 kw taps x 16 cin, weights [48, 32] per tap
    (host-preprocessed: shrink + hamming window; conv bias is dropped since
    training-mode BN cancels any per-channel constant shift).
  - PE column tiling: 4 strips process 4 h-blocks of 8 rows concurrently,
    PSUM tile [128, 512] = [(b,co), (h8,w64)] for one (d, h-half) tile.
  - Per-tile: ScalarE copies psum -> y_all SBUF (fp32), VectorE bn_stats.
  - bn_aggr -> per-partition (sum, sumsq) -> AllReduce[128,2] across 8 cores
    -> reduce over 4 h-blocks -> mean/istd -> per-partition scale/shift.
  - Phase 2: softplus(a*y+b) = Ln(1 + Exp(a*y+b)) on ScalarE, DMA out.
"""

import numpy as np
from contextlib import ExitStack

import concourse.bacc as bacc
import concourse.bass as bass
import concourse.tile as tile
from concourse import mybir
from concourse.bass_utils import run_bass_kernel_spmd

N, CIN, COUT, KK = 4, 16, 32, 3
D, H, W = 16, 64, 64
NCORES = 8
DSH = D // 2          # 8 output d-planes per core
HP, WP = H + 2, W + 2  # padded plane 66x66
PL = HP * WP           # 4356 elements per padded plane
NPLANES = DSH + 2      # 10 input planes per core
XBLEN = NPLANES * PL + 2
NTILES = DSH * 2       # (d, h-half) tiles
NCOL = 512             # psum free dim per tile
NPP = NTILES * NCOL    # positions per psum partition (8192)
NTOT = N * D * H * W   # BN population per channel (262144)
EPS = 1e-5

DT_MM = mybir.dt.float16


def _hamming(n):
    if n == 1:
        return np.ones((1,), np.float32)
    i = np.arange(n, dtype=np.float32)
    return (0.54 - 0.46 * np.cos(2.0 * np.float32(np.pi) * i / (n - 1))).astype(
        np.float32
    )


def preprocess_weights(weight):
    """shrink_conv_weights + hamming window, all fp32 numpy (matches reference)."""
    w = weight.astype(np.float32)
    cutoff = w.max(axis=(2, 3, 4), keepdims=True) * np.float32(0.5)
    shrunk = np.sign(w) * np.maximum(np.abs(w) - cutoff / np.float32(100.0), 0.0)
    w = np.where(w < cutoff, shrunk, w)
    win = (
        _hamming(KK)[:, None, None]
        * _hamming(KK)[None, :, None]
        * _hamming(KK)[None, None, :]
    )
    return (w * win[None, None]).astype(np.float32)


def build_w9(w):
    """w [COUT, CIN, 3,3,3] -> [9, 48, 32]: W9[kd*3+kh, kw*16+ci, co]."""
    w9 = np.transpose(w, (2, 3, 4, 1, 0))  # [kd, kh, kw, ci, co]
    return np.ascontiguousarray(w9.reshape(9, KK * CIN, COUT))


def build_xb(x_shard_padded):
    """[16, 10, 66, 66] fp32 -> [48, XBLEN] fp16 kw-triplicated flat buffer."""
    xf = x_shard_padded.reshape(CIN, -1)  # [16, 43560]
    L = xf.shape[1]
    xb = np.zeros((KK * CIN, XBLEN), np.float16)
    for kw in range(KK):
        xb[kw * CIN : (kw + 1) * CIN, : L - kw] = xf[:, kw:]
    return xb


def build_program():
    nc = bacc.Bacc(None, target_bir_lowering=False)
    xb_d = nc.dram_tensor("xb", [KK * CIN, XBLEN], DT_MM, kind="ExternalInput")
    w9_d = nc.dram_tensor("w9", [9, KK * CIN, COUT], DT_MM, kind="ExternalInput")
    gb_d = nc.dram_tensor("gb", [2, COUT], mybir.dt.float32, kind="ExternalInput")
    y_d = nc.dram_tensor("y", [COUT, DSH, H, W], mybir.dt.float32, kind="ExternalOutput")

    f32 = mybir.dt.float32
    with tile.TileContext(nc) as tc:
        with ExitStack() as ctx:
            singles = ctx.enter_context(tc.tile_pool(name="singles", bufs=1))
            xpool = ctx.enter_context(tc.tile_pool(name="xplanes", bufs=4))
            psum = ctx.enter_context(tc.tile_pool(name="psum", bufs=2, space="PSUM"))
            p2 = ctx.enter_context(tc.tile_pool(name="p2", bufs=3))
            small = ctx.enter_context(tc.tile_pool(name="small", bufs=2))
            dram = ctx.enter_context(tc.tile_pool(name="dram", bufs=2, space="DRAM"))

            w_sb = singles.tile([KK * CIN, 9, COUT], DT_MM)
            nc.sync.dma_start(out=w_sb, in_=w9_d[:, :, :].rearrange("r p m -> p r m"))
            gb_sb = singles.tile([COUT, 2], f32)
            nc.sync.dma_start(out=gb_sb, in_=gb_d[:, :].rearrange("j c -> c j"))
            eps_sb = singles.tile([COUT, 1], f32)
            nc.vector.memset(eps_sb, EPS)

            y_all = singles.tile([128, NTILES * NCOL], f32)
            stats_all = singles.tile([128, NTILES, 6], f32)

            planes = [None] * NPLANES

            def load_plane(p):
                planes[p] = xpool.tile(
                    [KK * CIN, PL], DT_MM, tag="plane", name=f"plane{p}"
                )
                nc.sync.dma_start(
                    out=planes[p], in_=xb_d[:, p * PL : (p + 1) * PL]
                )

            load_plane(0)
            load_plane(1)

            taps = [(kd, kh) for kd in range(KK) for kh in range(KK)]
            for d in range(DSH):
                load_plane(d + 2)
                for hh in range(2):
                    t = d * 2 + hh
                    ps = psum.tile([128, NCOL], f32)
                    for r, (kd, kh) in enumerate(taps):
                        src = planes[d + kd][:, :].rearrange(
                            "p (h w) -> p h w", w=WP
                        )
                        for b in range(4):
                            h0 = hh * 32 + b * 8 + kh
                            nc.tensor.matmul(
                                ps[32 * b : 32 * b + 32, :],
                                lhsT=w_sb[:, r, :],
                                rhs=src[:, h0 : h0 + 8, 0:W],
                                start=(r == 0),
                                stop=(r == 8),
                                tile_position=(0, 32 * b),
                            )
                    nc.scalar.copy(
                        out=y_all[:, t * NCOL : (t + 1) * NCOL], in_=ps[:, :]
                    )
                    nc.vector.bn_stats(out=stats_all[:, t, :], in_=ps[:, :])

            # ---- global BN stats ----
            mv = small.tile([128, 2], f32)
            nc.vector.bn_aggr(out=mv, in_=stats_all[:, :, :])
            # s1 = mean * NPP ; s2 = (var + mean^2) * NPP
            sq = small.tile([128, 1], f32)
            nc.vector.tensor_mul(sq, mv[:, 0:1], mv[:, 0:1])
            s2t = small.tile([128, 1], f32)
            nc.vector.tensor_add(s2t, mv[:, 1:2], sq)
            s12 = small.tile([128, 2], f32)
            nc.vector.tensor_scalar_mul(s12[:, 0:1], mv[:, 0:1], float(NPP))
            nc.vector.tensor_scalar_mul(s12[:, 1:2], s2t, float(NPP))

            cin_t = dram.tile([128, 2], f32)
            cout_t = dram.tile([128, 2], f32)
            nc.sync.dma_start(out=cin_t[:], in_=s12)
            nc.gpsimd.collective_compute(
                "AllReduce",
                mybir.AluOpType.add,
                replica_groups=[list(range(NCORES))],
                ins=[cin_t.opt()],
                outs=[cout_t.opt()],
            )
            # fetch as [co, j, b] so the innermost reduce sums the 4 h-blocks
            gsum = small.tile([COUT, 2, 4], f32)
            car = cout_t[:, :]
            nc.sync.dma_start(
                out=gsum,
                in_=bass.AP(
                    tensor=car.tensor,
                    offset=car.offset,
                    ap=[[2, COUT], [1, 2], [2 * COUT, 4]],
                ),
            )
            red = small.tile([COUT, 2], f32)
            nc.vector.tensor_reduce(
                out=red, in_=gsum, axis=mybir.AxisListType.X, op=mybir.AluOpType.add
            )
            mean_t = small.tile([COUT, 1], f32)
            nc.vector.tensor_scalar_mul(mean_t, red[:, 0:1], 1.0 / NTOT)
            ex2_t = small.tile([COUT, 1], f32)
            nc.vector.tensor_scalar_mul(ex2_t, red[:, 1:2], 1.0 / NTOT)
            msq_t = small.tile([COUT, 1], f32)
            nc.vector.tensor_mul(msq_t, mean_t, mean_t)
            var_t = small.tile([COUT, 1], f32)
            nc.vector.tensor_scalar(
                out=var_t,
                in0=ex2_t,
                scalar1=msq_t,
                scalar2=None,
                op0=mybir.AluOpType.subtract,
            )
            # istd = exp(-0.5 * ln(var + eps))
            lnv = small.tile([COUT, 1], f32)
            nc.scalar.activation(
                out=lnv,
                in_=var_t,
                func=mybir.ActivationFunctionType.Ln,
                bias=eps_sb[:, 0:1],
            )
            istd = small.tile([COUT, 1], f32)
            nc.scalar.activation(
                out=istd, in_=lnv, func=mybir.ActivationFunctionType.Exp, scale=-0.5
            )
            a_t = small.tile([COUT, 1], f32)
            nc.vector.tensor_mul(a_t, istd, gb_sb[:, 0:1])
            ma_t = small.tile([COUT, 1], f32)
            nc.vector.tensor_mul(ma_t, mean_t, a_t)
            ab32 = small.tile([COUT, 2], f32)
            nc.vector.tensor_copy(out=ab32[:, 0:1], in_=a_t)
            nc.vector.tensor_scalar(
                out=ab32[:, 1:2],
                in0=ma_t,
                scalar1=-1.0,
                scalar2=gb_sb[:, 1:2],
                op0=mybir.AluOpType.mult,
                op1=mybir.AluOpType.add,
            )
            # broadcast [32,2] -> [128,2] via DRAM bounce
            ab_dram = dram.tile([COUT, 2], f32)
            nc.sync.dma_start(out=ab_dram[:], in_=ab32)
            ab_sb = small.tile([128, 2], f32)
            abd = ab_dram[:, :]
            nc.sync.dma_start(
                out=ab_sb,
                in_=bass.AP(
                    tensor=abd.tensor,
                    offset=abd.offset,
                    ap=[[0, 4], [2, COUT], [1, 2]],
                ),
            )

            # ---- phase 2: softplus(a*y + b), store ----
            yh = y_d[:, :, :, :]
            for d in range(DSH):
                for hh in range(2):
                    t = d * 2 + hh
                    yn = p2.tile([128, NCOL], f32)
                    nc.scalar.activation(
                        out=yn,
                        in_=y_all[:, t * NCOL : (t + 1) * NCOL],
                        func=mybir.ActivationFunctionType.Exp,
                        scale=ab_sb[:, 0:1],
                        bias=ab_sb[:, 1:2],
                    )
                    nc.scalar.activation(
                        out=yn, in_=yn, func=mybir.ActivationFunctionType.Identity,
                        bias=1.0,
                    )
                    nc.scalar.activation(
                        out=yn, in_=yn, func=mybir.ActivationFunctionType.Ln
                    )
                    nc.sync.dma_start(
                        out=bass.AP(
                            tensor=yh.tensor,
                            offset=d * H * W + hh * 32 * W,
                            ap=[[8 * W, 4], [DSH * H * W, COUT], [W, 8], [1, W]],
                        ),
                        in_=yn,
                    )
    nc.finalize()
    return nc


_PROGRAM = None


def _get_program():
    global _PROGRAM
    if _PROGRAM is None:
        _PROGRAM = build_program()
    return _PROGRAM


def make_inputs(x, weight, gamma, beta):
    w = preprocess_weights(weight)
    w9 = build_w9(w).astype(np.float16)
    gb = np.stack([gamma.astype(np.float32), beta.astype(np.float32)], 0)
    x = np.asarray(x, np.float32)
    in_maps = []
    for c in range(NCORES):
        n, dh = c // 2, c % 2
        d0 = dh * DSH
        xp = np.zeros((CIN, NPLANES, HP, WP), np.float32)
        lo, hi = d0 - 1, d0 + DSH + 1
        slo, shi = max(lo, 0), min(hi, D)
        xp[:, slo - lo : shi - lo, 1 : H + 1, 1 : W + 1] = x[n, :, slo:shi]
        in_maps.append({"xb": build_xb(xp), "w9": w9, "gb": gb})
    return in_maps


def kernel(x, weight, bias, gamma, beta):
    nc = _get_program()
    in_maps = make_inputs(x, weight, gamma, beta)
    res = run_bass_kernel_spmd(nc, in_maps, list(range(NCORES)))
    out = np.empty((N, COUT, D, H, W), np.float32)
    for c in range(NCORES):
        n, dh = c // 2, c % 2
        out[n, :, dh * DSH : (dh + 1) * DSH] = res.results[c]["y"]
    return out
